# revision 32
# baseline (speedup 1.0000x reference)
"""Trainium2 Bass kernel for nn_DSBlock (diff_pool -> DGCNN -> diff_unpool).

Data-parallel over batch B=16 across 8 NeuronCores (2 batch elements per
core).  BatchNorm batch statistics are exchanged with three tiny
AllReduces (input-BN, DGCNN BN1, DGCNN BN2).

Fast path ("fold", engaged when all BN gammas > 0 and betas == 0, which
holds for this module): relu(BN(IN(x))) == A * relu(x - mean) with A > 0
per-channel, so h = relu(x - mean) is computed before the first
AllReduce and A is folded into per-batch copies of the dp/du projection
weights -- the AllReduce only gates a tiny [128,256] weight scaling.

Pool branch computes embed in [k, n] layout (stationary weight halves,
512-wide moving operand), takes exp with accum_out giving the softmax
denominator for free, and DMA-transposes E quarter-wise for the
x @ E^T contraction.

Self-contained: hardcodes shapes from the problem spec.
"""
import sys
sys.path.insert(0, '/opt/trn_rl_repo')

import numpy as np
import ml_dtypes

import concourse.bass as bass
import concourse.bacc as bacc
import concourse.tile as tile
from concourse import mybir
from concourse.bass_utils import run_bass_kernel_spmd
from concourse.alu_op_type import AluOpType

B, C, N, K = 16, 128, 8192, 256
NCORES, BLOC = 8, 2
INE, BNE = 1e-3, 1e-5
NT = N // 512          # 16 n-tiles of 512
NCH = N // 128         # 64 n-chunks of 128
f32, bf16 = mybir.dt.float32, mybir.dt.bfloat16
AF = mybir.ActivationFunctionType
ALU = AluOpType
BIG_NEG = -1.0e9

_cache: dict = {}
TRACE = False
TRACE_KW: dict = {}
ITERS = 1
LAST_RESULT = None


def _emit(nc, tc, cfg, iters=1):
    # ---------------- DRAM I/O ----------------
    xb_d = nc.dram_tensor("xb", [BLOC, C, N], bf16, kind="ExternalInput").ap()
    xbT_d = nc.dram_tensor("xbT", [128, BLOC, NCH, 128], bf16, kind="ExternalInput").ap()
    dpw_d = nc.dram_tensor("dpw_t", [C, K], bf16, kind="ExternalInput").ap()
    duw_d = nc.dram_tensor("duw_t", [C, K], bf16, kind="ExternalInput").ap()
    scA_d = nc.dram_tensor("scA_t", [C, C], bf16, kind="ExternalInput").ap()
    scB_d = nc.dram_tensor("scB_t", [C, C], bf16, kind="ExternalInput").ap()
    w1s_d = nc.dram_tensor("w1s_t", [K, K], bf16, kind="ExternalInput").ap()
    w1b_d = nc.dram_tensor("w1b_t", [K, K], bf16, kind="ExternalInput").ap()
    w2_d = nc.dram_tensor("w2_t", [K, K], bf16, kind="ExternalInput").ap()
    idf_d = nc.dram_tensor("identf", [128, 128], f32, kind="ExternalInput").ap()
    idb_d = nc.dram_tensor("identb", [128, 128], bf16, kind="ExternalInput").ap()
    pp_d = nc.dram_tensor("pp", [128, 13], f32, kind="ExternalInput").ap()
    dub_d = nc.dram_tensor("dub_bc", [128, K], f32, kind="ExternalInput").ap()
    out_d = nc.dram_tensor("out", [BLOC, C, N], bf16, kind="ExternalOutput").ap()
    for _it in range(iters):
        _emit_iter(nc, tc, cfg, xb_d, xbT_d, dpw_d, duw_d, scA_d, scB_d,
                   w1s_d, w1b_d, w2_d, idf_d, idb_d, pp_d, dub_d, out_d)


def _emit_iter(nc, tc, cfg, xb_d, xbT_d, dpw_d, duw_d, scA_d, scB_d,
               w1s_d, w1b_d, w2_d, idf_d, idb_d, pp_d, dub_d, out_d):
    fold, reuse_h, dub_zero, fold2 = cfg
    # ---------------- persistent pools ----------------
    import contextlib
    es = contextlib.ExitStack()
    consts = es.enter_context(tc.tile_pool(name="consts", bufs=1))
    bigp = es.enter_context(tc.tile_pool(name="bigp", bufs=2))
    small = es.enter_context(tc.tile_pool(name="small", bufs=1))
    dram = es.enter_context(tc.tile_pool(name="dram", bufs=1, space="DRAM"))

    # pool-phase-only pools (SBUF reclaimed afterwards)
    pool_sb = tc.tile_pool(name="pool_sb", bufs=2)
    psb = pool_sb.__enter__()
    xb = [bigp.tile([C, N], bf16, tag="xb", name=f"xb{b_}") for b_ in range(BLOC)]
    h = [bigp.tile([C, N], bf16, tag="h", name=f"h{b_}") for b_ in range(BLOC)]
    xT = [psb.tile([128, NCH, 128], bf16, tag="xT", name=f"xT{b_}") for b_ in range(BLOC)]

    # light consts first, then xT (gates IN stats), then ar1_i, then xb,
    # then the heavy consts -- this keeps the tiny AR1 input DMA from
    # queueing behind bulk transfers on the serial DMA engines.
    pp = consts.tile([128, 13], f32); nc.sync.dma_start(pp[:], pp_d[:])
    idf = consts.tile([128, 128], f32); nc.sync.dma_start(idf[:], idf_d[:])
    idb = consts.tile([128, 128], bf16); nc.sync.dma_start(idb[:], idb_d[:])
    for b in range(BLOC):
        for hf in range(2):
            nc.sync.dma_start(xT[b][:, hf * 32:(hf + 1) * 32, :],
                              xbT_d[:, b, hf * 32:(hf + 1) * 32, :])

    ones_f = consts.tile([128, 1], f32); nc.vector.memset(ones_f[:], 1.0)
    ones_b2 = consts.tile([128, 1], bf16); nc.vector.memset(ones_b2[:], 1.0)
    i32 = mybir.dt.int32
    magic = consts.tile([128, 4], i32); nc.vector.memset(magic[:], 0x5f3759df)
    rs_w = small.tile([128, 4], f32)

    def emit_rsqrt(out_ap, in_ap, W, eng=None):
        # out = rsqrt(in) via bit-trick seed + 2 Newton steps, all on DVE
        # (keeps sqrt/reciprocal activation tables off the Act engine)
        v = eng or nc.vector
        wi = rs_w[:, 0:W]
        v.tensor_scalar(wi.bitcast(i32), in_ap.bitcast(i32), 1, None,
                        ALU.arith_shift_right)
        v.tensor_tensor(out_ap.bitcast(i32), magic[:, 0:W], wi.bitcast(i32),
                        ALU.subtract)
        for _ in range(2):
            v.tensor_tensor(wi, out_ap, out_ap, ALU.mult)
            v.tensor_tensor(wi, wi, in_ap, ALU.mult)
            v.tensor_scalar(wi, wi, -0.5, 1.5, ALU.mult, ALU.add)
            v.tensor_tensor(out_ap, out_ap, wi, ALU.mult)

    DPG, DPB, DUG, DUB_, SCB = 0, 1, 2, 3, 4
    G1_, BB1, G2_, BB2 = 5, 7, 9, 11  # +h for half

    # ---------------- IN stats: sums + sumsq via PE ones-matmuls over xT ----
    mv = small.tile([128, BLOC, 2], f32)
    tvb = small.tile([128, BLOC], f32)   # v + INE
    prt = small.tile([128, BLOC + 1], f32)
    negm = small.tile([128, BLOC], f32)
    rtv = small.tile([128, BLOC], f32)
    # prewarm the exp activation table while Act is idle
    warm = small.tile([128, 1], bf16)
    nc.scalar.activation(warm[:], ones_b2[:], AF.Exp)
    front_ps = tc.tile_pool(name="front_ps", bufs=1, space="PSUM")
    fps0 = front_ps.__enter__()
    smm = fps0.tile([1, 4, 128], f32, name="smm")    # [Sx_b0, Sx_b1, Sq_b0, Sq_b1]
    ztf = fps0.tile([128, 4], f32, name="ztf")
    with tc.tile_pool(name="sqp", bufs=2) as sqp:
        for b in range(BLOC):
            for hf in range(2):
                for a in range(hf * 32, hf * 32 + 32):
                    nc.tensor.matmul(smm[:, b, :], ones_b2[:], xT[b][:, a, :],
                                     start=(hf == 0 and a == 0),
                                     stop=(hf == 1 and a == hf * 32 + 31))
                for s2_ in range(2):
                    a0 = hf * 32 + s2_ * 16
                    sq = sqp.tile([128, 16, 128], bf16, tag="sq", bufs=3,
                                  name=f"sq_{b}_{hf}_{s2_}")
                    nc.vector.tensor_tensor(
                        sq[:].rearrange("p a c -> p (a c)"),
                        xT[b][:, a0:a0 + 16, :].rearrange("p a c -> p (a c)"),
                        xT[b][:, a0:a0 + 16, :].rearrange("p a c -> p (a c)"),
                        ALU.mult)
                    for j in range(16):
                        nc.tensor.matmul(smm[:, 2 + b, :], ones_b2[:], sq[:, j, :],
                                         start=(hf == 0 and s2_ == 0 and j == 0),
                                         stop=(hf == 1 and s2_ == 1 and j == 15))
    smm_s = small.tile([1, 4, 128], f32)
    nc.vector.tensor_copy(smm_s[:], smm[:])
    for i in range(4):
        nc.tensor.transpose(ztf[:, i:i + 1], smm_s[0:1, i, :], idf[0:1, 0:1])
    v = nc.vector
    v.tensor_scalar(mv[:, :, 0], ztf[:, 0:2], 1.0 / N, None, ALU.mult)   # means
    v.tensor_scalar(mv[:, :, 1], ztf[:, 2:4], 1.0 / N, None, ALU.mult)   # E[x^2]
    v.tensor_tensor(rtv[:], mv[:, :, 0], mv[:, :, 0], ALU.mult)          # m^2
    v.tensor_tensor(mv[:, :, 1], mv[:, :, 1], rtv[:], ALU.subtract)      # var
    v.tensor_scalar(tvb[:], mv[:, :, 1], INE, None, ALU.add)
    v.tensor_scalar(negm[:], mv[:, :, 0], -1.0, None, ALU.mult)
    v.reciprocal(rtv[:], tvb[:])
    v.tensor_tensor(prt[:, 0:BLOC], mv[:, :, 1], rtv[:], ALU.mult)
    v.tensor_tensor(prt[:, BLOC:BLOC + 1], prt[:, 0:1], prt[:, 1:2], ALU.add)
    front_ps.__exit__(None, None, None)
    ar1_i = dram.tile([128, 1], f32)
    ar1_o = dram.tile([NCORES, 128, 1], f32)
    nc.sync.dma_start(ar1_i[:], prt[:, BLOC:BLOC + 1])
    nc.gpsimd.collective_compute("AllGather", mybir.AluOpType.bypass,
                                 replica_groups=[list(range(NCORES))],
                                 ins=[ar1_i.opt()], outs=[ar1_o.opt()])

    # xb + heavy consts load after the AR1 input DMA
    for b in range(BLOC):
        for q in range(4):
            qs = slice(q * 2048, (q + 1) * 2048)
            nc.sync.dma_start(xb[b][:, qs], xb_d[b][:, qs])
    dpw = consts.tile([C, K], bf16); nc.sync.dma_start(dpw[:], dpw_d[:])
    duw = consts.tile([C, K], bf16); nc.sync.dma_start(duw[:], duw_d[:])
    scA = consts.tile([C, C], bf16); nc.sync.dma_start(scA[:], scA_d[:])
    scB = consts.tile([C, C], bf16); nc.sync.dma_start(scB[:], scB_d[:])
    w1s = consts.tile([128, 2, K], bf16)
    nc.sync.dma_start(w1s[:], w1s_d.rearrange("(h c) o -> c h o", h=2))
    w1b = consts.tile([128, 2, K], bf16)
    nc.sync.dma_start(w1b[:], w1b_d.rearrange("(h c) o -> c h o", h=2))
    w2 = consts.tile([128, 2, K], bf16)
    nc.sync.dma_start(w2[:], w2_d.rearrange("(h c) o -> c h o", h=2))
    if not dub_zero:
        dub = consts.tile([128, K], f32); nc.sync.dma_start(dub[:], dub_d[:])

    if fold:
        # h = relu(x - m) on DVE (bf16 4x tensor_scalar), fills the AG1 window
        for b in range(BLOC):
            for t in range(4):
                sl = slice(t * 2048, (t + 1) * 2048)
                nc.vector.tensor_scalar(h[b][:, sl], xb[b][:, sl],
                                        negm[:, b:b + 1], 0.0, ALU.add, ALU.max)
        h2 = h
        # local per-b gamma*rsqrt(v+INE) factors, ready before AG1 lands
        Ldp = small.tile([128, BLOC], f32); Ldu = small.tile([128, BLOC], f32)
        st_ = small.tile([128, BLOC], f32)
        emit_rsqrt(st_[:, 0:BLOC], tvb[:, 0:BLOC], BLOC)
        for b in range(BLOC):
            nc.gpsimd.tensor_tensor(Ldp[:, b:b + 1], pp[:, DPG:DPG + 1],
                                    st_[:, b:b + 1], ALU.mult)
            nc.gpsimd.tensor_tensor(Ldu[:, b:b + 1], pp[:, DUG:DUG + 1],
                                    st_[:, b:b + 1], ALU.mult)

    arv8 = small.tile([128, NCORES], f32)
    nc.scalar.dma_start(arv8[:], ar1_o[:].rearrange("d p o -> p (d o)"))
    arv = small.tile([128, 1], f32)
    nc.vector.tensor_reduce(arv[:], arv8[:], mybir.AxisListType.X, ALU.add)

    # BN-in affine per (b, branch)
    vb2 = small.tile([128, 1], f32)   # var_c + BNE
    nc.vector.tensor_scalar(vb2[:], arv[:], 1.0 / B, BNE, ALU.mult, ALU.add)
    if fold:
        # R = rsqrt(var+eps) shared across b and branches
        R_ = small.tile([128, 3], f32)
        emit_rsqrt(R_[:, 1:2], vb2[:], 1)
        dpwb = [consts.tile([C, K], bf16, name=f"dpwb{b_}") for b_ in range(BLOC)]
        duwb = [consts.tile([C, K], bf16, name=f"duwb{b_}") for b_ in range(BLOC)]
        Adp = small.tile([128, BLOC], f32)
        Adu = small.tile([128, BLOC], f32)
        for b in range(BLOC):
            nc.vector.tensor_tensor(Adp[:, b:b + 1], R_[:, 1:2], Ldp[:, b:b + 1], ALU.mult)
            nc.vector.tensor_scalar(dpwb[b][:], dpw[:], Adp[:, b:b + 1], None, ALU.mult)
            nc.vector.tensor_tensor(Adu[:, b:b + 1], R_[:, 1:2], Ldu[:, b:b + 1], ALU.mult)
            nc.vector.tensor_scalar(duwb[b][:], duw[:], Adu[:, b:b + 1], None, ALU.mult)
    else:
        Adp = small.tile([128, BLOC], f32); Bdp = small.tile([128, BLOC], f32)
        Adu = small.tile([128, BLOC], f32); Bdu = small.tile([128, BLOC], f32)
        wrk = small.tile([128, 4], f32)
        for b in range(BLOC):
            nc.vector.tensor_tensor(wrk[:, 0:1], vb2[:], tvb[:, b:b + 1], ALU.mult)
            nc.scalar.sqrt(wrk[:, 1:2], wrk[:, 0:1])
            nc.vector.reciprocal(wrk[:, 0:1], wrk[:, 1:2])   # R_b
            nc.vector.tensor_tensor(Adp[:, b:b + 1], wrk[:, 0:1], pp[:, DPG:DPG + 1], ALU.mult)
            if not reuse_h:
                nc.vector.tensor_tensor(Adu[:, b:b + 1], wrk[:, 0:1], pp[:, DUG:DUG + 1], ALU.mult)
            nc.vector.tensor_scalar(wrk[:, 2:3], mv[:, b, 0:1], -1.0, None, ALU.mult)  # -m
            nc.vector.scalar_tensor_tensor(Bdp[:, b:b + 1], Adp[:, b:b + 1], wrk[:, 2:3],
                                           pp[:, DPB:DPB + 1], ALU.mult, ALU.add)
            if not reuse_h:
                nc.vector.scalar_tensor_tensor(Bdu[:, b:b + 1], Adu[:, b:b + 1], wrk[:, 2:3],
                                               pp[:, DUB_:DUB_ + 1], ALU.mult, ALU.add)

    if not fold:
        dpwb = [dpw, dpw]
        duwb = [duw, duw]
        for b in range(BLOC):
            for t in range(8):
                sl = slice(t * 1024, (t + 1) * 1024)
                nc.scalar.activation(h[b][:, sl], xb[b][:, sl], AF.Relu,
                                     bias=Bdp[:, b:b + 1], scale=Adp[:, b:b + 1])
        if reuse_h:
            h2 = h
        else:
            h2 = [bigp.tile([C, N], bf16, tag="h2", name=f"h2_{b_}") for b_ in range(BLOC)]
            for b in range(BLOC):
                for t in range(8):
                    sl = slice(t * 1024, (t + 1) * 1024)
                    nc.scalar.activation(h2[b][:, sl], xb[b][:, sl], AF.Relu,
                                         bias=Bdu[:, b:b + 1], scale=Adu[:, b:b + 1])

    # ---------------- pool branch ----------------
    # E^T computed directly in [n, k] layout (stationary = h 128-chunks,
    # moving = dpwb), so the x @ E^T contraction needs NO transposes.
    # Z[k] via ones-stationary matmuls over the eT tiles (moving side),
    # folded + PE-transposed to [k-part] at the end of each b.
    feats_f = []   # [2][128,128] f32 per b (k-half, c)
    feats_b = []   # bf16
    f2x_f = []     # 2*feats f32
    ones_b2 = consts.tile([128, 1], bf16); nc.vector.memset(ones_b2[:], 1.0)
    with (
        tc.tile_pool(name="ph_a_ps", bufs=1, space="PSUM") as aps,
        tc.tile_pool(name="etp", bufs=3) as etp,
    ):
        def emit_eT(b, g):
            # PE: 4 chunk matmuls into psum; Act: one 1024-wide exp
            ps = aps.tile([128, 4, K], f32, tag="eps", bufs=2, name=f"ps_{b}_{g}")
            for j in range(4):
                a = g * 4 + j
                nc.tensor.matmul(ps[:, j, :], h[b][:, a * 128:(a + 1) * 128],
                                 dpwb[b][:])
            eT = etp.tile([128, 4, K], bf16, tag="eT", bufs=3, name=f"eT_{b}_{g}")
            nc.scalar.activation(eT[:].rearrange("p a k -> p (a k)"),
                                 ps[:].rearrange("p a k -> p (a k)"), AF.Exp)
            return eT

        def emit_consume(b, g, eT, xd, zps):
            for j in range(4):
                a = g * 4 + j
                nc.tensor.matmul(xd[:], xT[b][:, a, :], eT[:, j, :],
                                 start=(a == 0), stop=(a == NCH - 1))
            for z2 in range(2):
                nc.tensor.matmul(
                    zps[:], ones_b2[:],
                    eT[:, z2 * 2:(z2 + 1) * 2, :].rearrange("p a k -> p (a k)"),
                    start=(g == 0 and z2 == 0), stop=(g == 15 and z2 == 1))

        for b in range(BLOC):
            xd = aps.tile([128, K], f32, tag="xd", bufs=1, name=f"xd_{b}")
            zps = aps.tile([1, 512], f32, tag="zps", bufs=1, name=f"zps_{b}")
            # software pipeline: eT of group g+1 is emitted before the
            # consumers of group g so PE never head-blocks on the Act exp
            eTs = {0: emit_eT(b, 0)}
            for g in range(16):
                if g + 1 < 16:
                    eTs[g + 1] = emit_eT(b, g + 1)
                emit_consume(b, g, eTs.pop(g), xd, zps)
            # Z: fold j-pairs, transpose to [k-part], reciprocal
            zrow = psb.tile([1, 512], f32, tag="zrow", name=f"zrow_{b}")
            nc.vector.tensor_copy(zrow[:], zps[:])
            nc.vector.tensor_tensor(zrow[:, 0:256], zrow[:, 0:256],
                                    zrow[:, 256:512], ALU.add)
            rc = small.tile([128, 4], f32, tag=f"rc_{b}", name=f"rc_{b}")
            ztp = aps.tile([128, 2], f32, tag="ztp", bufs=1, name=f"ztp_{b}")
            for hh in range(2):
                nc.tensor.transpose(ztp[:, hh:hh + 1],
                                    zrow[:, hh * 128:(hh + 1) * 128],
                                    idf[0:1, 0:1])
            nc.vector.reciprocal(rc[:, 0:2], ztp[:])
            nc.vector.tensor_scalar(rc[:, 2:4], rc[:, 0:2], 2.0, None, ALU.mult)
            xd_s = psb.tile([128, K], f32, tag="xds", name=f"xds_{b}")
            nc.vector.tensor_copy(xd_s[:], xd[:])
            ff = []; fb = []; f2 = []
            for hh in range(2):
                tp = aps.tile([128, 128], f32, tag="tp", bufs=1)
                nc.tensor.transpose(tp[:], xd_s[:, hh * 128:(hh + 1) * 128], idf[:])
                t_f = small.tile([128, 128], f32, tag=f"ff_{b}_{hh}")
                t_b2 = small.tile([128, 128], bf16, tag=f"fb_{b}_{hh}")
                t_2x = small.tile([128, 128], f32, tag=f"f2_{b}_{hh}")
                nc.vector.tensor_scalar(t_f[:], tp[:], rc[:, hh:hh + 1], None, ALU.mult)
                nc.vector.tensor_scalar(t_b2[:], tp[:], rc[:, hh:hh + 1], None, ALU.mult)
                nc.vector.tensor_scalar(t_2x[:], tp[:], rc[:, 2 + hh:3 + hh], None, ALU.mult)
                ff.append(t_f); fb.append(t_b2); f2.append(t_2x)
            feats_f.append(ff); feats_b.append(fb); f2x_f.append(f2)
    pool_sb.__exit__(None, None, None)

    s2t_p = es.enter_context(tc.tile_pool(name="s2t", bufs=6))
    s2_p = es.enter_context(tc.tile_pool(name="s2", bufs=32))
    outp = es.enter_context(tc.tile_pool(name="outp", bufs=8))

    # ---------------- DGCNN part A: knn prep (pd/sq/GT/C) ----------------
    c1 = [[small.tile([128, 6, 128], bf16, tag=f"c1_{b}_{oh}", name=f"c1_{b}_{oh}") for oh in range(2)]
          for b in range(BLOC)]
    st1 = small.tile([128, 2 * BLOC, 2, 6], f32)
    pd_sl = [small.tile([128, 128], f32, tag=f"pds_{b}", name=f"pds_{b}") for b in range(BLOC)]
    mx8l = [small.tile([128, 8], f32, tag=f"mx8_{b}", name=f"mx8_{b}") for b in range(BLOC)]
    GTl = [small.tile([128, K], bf16, tag=f"GT_{b}", name=f"GT_{b}") for b in range(BLOC)]
    C_sl = [small.tile([128, 2, 128], f32, tag=f"Cs_{b}", name=f"Cs_{b}") for b in range(BLOC)]
    with (
        tc.tile_pool(name="ph_b_ps", bufs=1, space="PSUM") as bps,
        tc.tile_pool(name="ph_b_sb", bufs=2) as bsb,
    ):
        for b in range(BLOC):
            # sq[p] via ones @ feats^2
            fsq = bsb.tile([128, 2, 128], f32, tag="fsq")
            for hh in range(2):
                nc.vector.tensor_tensor(fsq[:, hh, :], feats_f[b][hh][:],
                                        feats_f[b][hh][:], ALU.mult)
            sqp = bps.tile([1, 128], f32, tag="sq", bufs=2)
            for hh in range(2):
                nc.tensor.matmul(sqp[:], ones_f[:], fsq[:, hh, :],
                                 start=(hh == 0), stop=(hh == 1))
            negsq = bsb.tile([1, 128], f32, tag="negsq")
            nc.vector.tensor_scalar(negsq[:], sqp[:], -1.0, None, ALU.mult)
            onesr = bsb.tile([1, 128], f32, tag="onesr")
            nc.vector.memset(onesr[:], 1.0)
            pdp = bps.tile([128, 128], f32, tag="pd", bufs=2)
            for hh in range(2):
                nc.tensor.matmul(pdp[:], f2x_f[b][hh][:], feats_f[b][hh][:],
                                 start=(hh == 0), stop=False)
            nc.tensor.matmul(pdp[:], negsq[:], onesr[:], start=False, stop=False)
            nc.tensor.matmul(pdp[:], onesr[:], negsq[:], start=False, stop=True)
            nc.vector.tensor_copy(pd_sl[b][:], pdp[:])
            # G^T = f^T W1b^T   [p, o=256]
            gtp = bps.tile([128, K], f32, tag="gt", bufs=2)
            for hh in range(2):
                nc.tensor.matmul(gtp[:], feats_b[b][hh][:], w1b[:, hh, :],
                                 start=(hh == 0), stop=(hh == 1))
            nc.vector.tensor_copy(GTl[b][:], gtp[:])
            # C = W1s @ f   [o-half, p] x2
            for oh in range(2):
                cp = bps.tile([128, 128], f32, tag="Cp", bufs=2)
                for hh in range(2):
                    nc.tensor.matmul(cp[:], w1s[:, hh, oh * 128:(oh + 1) * 128],
                                     feats_b[b][hh][:], start=(hh == 0), stop=(hh == 1))
                nc.vector.tensor_copy(C_sl[b][:, oh, :], cp[:])
            nc.vector.max(mx8l[b][:], pd_sl[b][:])

    def emit_masks(b, kps, ksb, is_eq_eng):
        # 6 independent equality masks (distances are distinct floats)
        pd_s, mx8, GT, C_s = pd_sl[b], mx8l[b], GTl[b], C_sl[b]
        for j in range(6):
            mk = ksb.tile([128, 128], bf16, tag="mk", bufs=3)
            is_eq_eng.tensor_scalar(mk[:], pd_s[:], mx8[:, j:j + 1], None, ALU.is_equal)
            mtp = kps.tile([128, 128], bf16, tag="kmtp", bufs=1)
            nc.tensor.transpose(mtp[:], mk[:], idb[:])
            mkT = ksb.tile([128, 128], bf16, tag="mkT", bufs=3)
            nc.vector.tensor_copy(mkT[:], mtp[:])
            for oh in range(2):
                ntp = kps.tile([128, 128], f32, tag="kntp", bufs=1)
                nc.tensor.matmul(ntp[:], GT[:, oh * 128:(oh + 1) * 128], mkT[:])
                nc.vector.scalar_tensor_tensor(c1[b][oh][:, j, :], ntp[:], -1.0,
                                               C_s[:, oh, :], ALU.mult, ALU.add)
        for oh in range(2):
            for ch in range(2):
                nc.vector.bn_stats(st1[:, 2 * b + oh, ch, :],
                                   c1[b][oh][:, :, :].rearrange("p a b -> p (a b)")[:, ch * 384:(ch + 1) * 384])

    # ---------------- unpool machinery ----------------
    s2grp = {}
    s2tgrp = {}
    zk = [small.tile([128, NCH], f32, tag=f"zk_{b}", name=f"zk_{b}") for b in range(BLOC)]
    rcp2 = [small.tile([128, NCH], f32, tag=f"rk_{b}", name=f"rk_{b}") for b in range(BLOC)]

    def emit_normalize_group(b, g, scale_eng=None):
        s2t = s2tgrp[(b, g)]
        for q in range(4):
            a = g * 4 + q
            nc.vector.tensor_reduce(zk[b][:, a:a + 1], s2t[:, q, :],
                                    mybir.AxisListType.X, ALU.add)
            nc.vector.reciprocal(rcp2[b][:, a:a + 1], zk[b][:, a:a + 1])
            eng = scale_eng if scale_eng is not None else (
                nc.vector if q % 2 == 0 else nc.gpsimd)
            eng.tensor_scalar(s2t[:, q, :], s2t[:, q, :],
                              rcp2[b][:, a:a + 1], None, ALU.mult)

    def emit_exp_group(b, g, e2ps_pool, scr_pool, normalize=True, scale_eng=None,
                       accum=False):
        s2t = s2t_p.tile([128, 4, K], bf16, tag="s2t", name=f"s2t_{b}_{g}")
        s2tgrp[(b, g)] = s2t
        for half in range(2):
            ep = e2ps_pool.tile([128, 512], f32, tag="e2", bufs=2,
                                name=f"ep_{b}_{g}_{half}")
            for qq in range(2):
                a = g * 4 + half * 2 + qq
                nc.tensor.matmul(ep[:, qq * 256:(qq + 1) * 256],
                                 h2[b][:, a * 128:(a + 1) * 128], duwb[b][:])
            if dub_zero:
                src = ep
            else:
                src = scr_pool.tile([128, 512], f32, tag="e2s",
                                    name=f"sc_{b}_{g}_{half}")
                for qq in range(2):
                    nc.vector.tensor_tensor(src[:, qq * 256:(qq + 1) * 256],
                                            ep[:, qq * 256:(qq + 1) * 256],
                                            dub[:], ALU.add)
            if accum:
                # 256-wide exps with accum_out: zk comes free from the Act
                # engine, keeping DVE clear for the st2 -> AR3 chain
                for qq in range(2):
                    a = g * 4 + half * 2 + qq
                    nc.scalar.activation(s2t[:, half * 2 + qq, :],
                                         src[:, qq * 256:(qq + 1) * 256], AF.Exp,
                                         accum_out=zk[b][:, a:a + 1])
                    nc.vector.reciprocal(rcp2[b][:, a:a + 1], zk[b][:, a:a + 1])
                    eng = scale_eng if scale_eng is not None else nc.gpsimd
                    eng.tensor_scalar(s2t[:, half * 2 + qq, :],
                                      s2t[:, half * 2 + qq, :],
                                      rcp2[b][:, a:a + 1], None, ALU.mult)
            else:
                nc.scalar.activation(
                    s2t[:, half * 2:(half + 1) * 2, :].rearrange("p a b -> p (a b)"),
                    src[:], AF.Exp)
        if not accum and normalize:
            emit_normalize_group(b, g, scale_eng)

    def emit_transpose_group(b, g):
        s2 = s2_p.tile([128, 8, 128], bf16, tag="s2", name=f"s2_{b}_{g}")
        nc.sync.dma_start_transpose(s2[:], s2tgrp[(b, g)][:])
        s2grp[(b, g)] = s2

    def emit_prestage(stg_pool):
        # scA @ x + sc_b staged in place into the (otherwise dead) xb tiles,
        # so the post-AR3 tail only needs the y2@S2 matmuls + one add.
        i = 0
        for b in range(BLOC):
            for t in range(NT):
                sl = slice(t * 512, (t + 1) * 512)
                sp = stg_pool.tile([128, 512], f32, tag="stg", bufs=2,
                                   name=f"stg_{b}_{t}")
                nc.tensor.matmul(sp[:], scA[:], xb[b][:, sl])
                if i % 2 == 0:
                    nc.vector.tensor_scalar(xb[b][:, sl], sp[:],
                                            pp[:, SCB:SCB + 1], None, ALU.add)
                else:
                    nc.scalar.activation(xb[b][:, sl], sp[:], AF.Identity,
                                         bias=pp[:, SCB:SCB + 1])
                i += 1

    def emit_final_block(b, ts, fpool):
        fps = [fpool.tile([128, 512], f32, tag="fps", bufs=8, name=f"fps_{b}_{t}")
               for t in ts]
        for i, t in enumerate(ts):
            if t % 2 == 0:
                nc.tensor.matmul(fps[i][:], idb[:],
                                 xb[b][:, t * 512:(t + 1) * 512],
                                 start=True, stop=False)
        for kh in range(2):
            for i, t in enumerate(ts):
                s2v = s2grp[(b, t)][:].rearrange("p (a k) q -> p k a q", k=2)
                nc.tensor.matmul(fps[i][:], y2[b * 2 + kh][:],
                                 s2v[:, kh, :, :],
                                 start=(kh == 0 and t % 2 == 1), stop=(kh == 1))
        for i, t in enumerate(ts):
            ot = outp.tile([128, 512], bf16, tag="ot", name=f"ot_{b}_{t}")
            if t % 2 == 0:
                nc.scalar.copy(ot[:], fps[i][:])
            else:
                nc.vector.tensor_tensor(ot[:], fps[i][:],
                                        xb[b][:, t * 512:(t + 1) * 512], ALU.add)
            nc.sync.dma_start(out_d[b][:, t * 512:(t + 1) * 512], ot[:])

    PRE1 = 12  # b=1 exp-only groups emitted before the BN1/conv2 section

    with (
        tc.tile_pool(name="ph_c_ps", bufs=1, space="PSUM") as cps,
        tc.tile_pool(name="ph_c_sb", bufs=2) as csb,
    ):
        e2ps, c2ps = cps, cps
        emit_masks(0, cps, csb, nc.gpsimd)
        emit_masks(1, cps, csb, nc.vector)
        # BN1 partial -> AR2
        mv1 = small.tile([128, 2, 2], f32)
        ar2b = small.tile([128, 4], f32)
        for oh in range(2):
            nc.vector.bn_aggr(mv1[:, oh, :], st1[:].rearrange("p (b o) c s -> p o b c s", o=2)[:, oh])
            nc.vector.tensor_scalar(ar2b[:, oh:oh + 1], mv1[:, oh, 0:1], 1536.0, None, ALU.mult)
            nc.vector.scalar_tensor_tensor(ar2b[:, 2 + oh:3 + oh], mv1[:, oh, 0:1],
                                           mv1[:, oh, 0:1], mv1[:, oh, 1:2], ALU.mult, ALU.add)
            nc.vector.tensor_scalar(ar2b[:, 2 + oh:3 + oh], ar2b[:, 2 + oh:3 + oh],
                                    1536.0, None, ALU.mult)
        ar2_i = dram.tile([128, 4], f32)
        ar2_o = dram.tile([NCORES, 128, 4], f32)
        nc.sync.dma_start(ar2_i[:], ar2b[:])
        nc.gpsimd.collective_compute("AllGather", mybir.AluOpType.bypass,
                                     replica_groups=[list(range(NCORES))],
                                     ins=[ar2_i.opt()], outs=[ar2_o.opt()])
        ar2g = small.tile([128, NCORES, 4], f32)
        nc.scalar.dma_start(ar2g[:], ar2_o[:].rearrange("d p f -> p d f"))
        ar2r = small.tile([128, 4], f32)
        nc.vector.tensor_reduce(ar2r[:], ar2g[:].rearrange("p d f -> p f d"),
                                mybir.AxisListType.X, ALU.add)

        # b0 groups fill the knn->AR2 stretch and the AR2 window
        for g in range(NT):
            emit_exp_group(0, g, e2ps, csb, accum=True,
                           scale_eng=nc.gpsimd)
            emit_transpose_group(0, g)

        # scA @ x prestage: PE/DVE/Act idle out the AR2 window here
        emit_prestage(cps)

        # -------- BN1 apply + conv2 + BN2 stats + AR3 --------
        a1 = small.tile([128, 2], f32); b1 = small.tile([128, 2], f32)
        wk2 = small.tile([128, 6], f32)
        CNT1 = float(B * 128 * 6)
        v = nc.vector
        v.tensor_scalar(wk2[:, 0:2], ar2r[:, 0:2], 1.0 / CNT1, None, ALU.mult)  # means
        v.tensor_scalar(wk2[:, 2:4], ar2r[:, 2:4], 1.0 / CNT1, None, ALU.mult)  # E[x^2]
        v.tensor_tensor(wk2[:, 4:6], wk2[:, 0:2], wk2[:, 0:2], ALU.mult)        # m^2
        v.tensor_tensor(wk2[:, 2:4], wk2[:, 2:4], wk2[:, 4:6], ALU.subtract)    # var
        v.tensor_scalar(wk2[:, 2:4], wk2[:, 2:4], BNE, None, ALU.add)           # var+eps
        emit_rsqrt(wk2[:, 4:6], wk2[:, 2:4], 2)
        v.tensor_tensor(a1[:, 0:2], wk2[:, 4:6], pp[:, G1_:G1_ + 2], ALU.mult)
        v.tensor_tensor(wk2[:, 0:2], wk2[:, 0:2], a1[:, 0:2], ALU.mult)         # m*a1
        v.tensor_tensor(b1[:, 0:2], pp[:, BB1:BB1 + 2], wk2[:, 0:2], ALU.subtract)
        g1 = [[csb.tile([128, 6, 128], bf16, tag=f"g1_{b}_{oh}", name=f"g1_{b}_{oh}", bufs=1) for oh in range(2)]
              for b in range(BLOC)]
        c2 = [[csb.tile([128, 6, 128], bf16, tag=f"c2_{b}_{oh}", name=f"c2_{b}_{oh}", bufs=1) for oh in range(2)]
              for b in range(BLOC)]
        st2 = small.tile([128, 2 * BLOC, 2, 6], f32)
        gmxr = {}
        for b in range(BLOC):
            for oh in range(2):
                nc.scalar.activation(g1[b][oh][:].rearrange("p a b -> p (a b)"),
                                     c1[b][oh][:].rearrange("p a b -> p (a b)"),
                                     AF.Relu, bias=b1[:, oh:oh + 1], scale=a1[:, oh:oh + 1])
            for oh in range(2):
                g1f = [g1[b][ch][:].rearrange("p a b -> p (a b)") for ch in range(2)]
                if fold2:
                    c2f = c2[b][oh][:].rearrange("p a b -> p (a b)")
                    for fh in range(2):
                        cp2 = c2ps.tile([128, 384], f32, tag="c2p", bufs=2,
                                        name=f"cp2_{b}_{oh}_{fh}")
                        for ch in range(2):
                            nc.tensor.matmul(cp2[:], w2[:, ch, oh * 128:(oh + 1) * 128],
                                             g1f[ch][:, fh * 384:(fh + 1) * 384],
                                             start=(ch == 0), stop=(ch == 1))
                        nc.scalar.copy(c2f[:, fh * 384:(fh + 1) * 384], cp2[:])
                    for ch in range(2):
                        nc.vector.bn_stats(st2[:, 2 * b + oh, ch, :],
                                           c2f[:, ch * 384:(ch + 1) * 384])
                    gmxr[(b, oh)] = csb.tile([128, 128], f32, tag="gmxr",
                                             name=f"gmxr_{b}_{oh}", bufs=4)
                    nc.vector.reduce_max(gmxr[(b, oh)][:],
                                         c2[b][oh][:].rearrange("p a b -> p b a"),
                                         mybir.AxisListType.X)
                else:
                    c2f = c2[b][oh][:].rearrange("p a b -> p (a b)")
                    for fh in range(2):
                        cp2 = c2ps.tile([128, 384], f32, tag="c2p", bufs=2, name=f"cp2_{b}_{oh}_{fh}")
                        for ch in range(2):
                            nc.tensor.matmul(cp2[:], w2[:, ch, oh * 128:(oh + 1) * 128],
                                             g1f[ch][:, fh * 384:(fh + 1) * 384],
                                             start=(ch == 0), stop=(ch == 1))
                        nc.scalar.copy(c2f[:, fh * 384:(fh + 1) * 384], cp2[:])
                    for ch in range(2):
                        nc.vector.bn_stats(st2[:, 2 * b + oh, ch, :],
                                           c2f[:, ch * 384:(ch + 1) * 384])
        mv2 = small.tile([128, 2, 2], f32)
        ar3b = small.tile([128, 4], f32)
        for oh in range(2):
            nc.vector.bn_aggr(mv2[:, oh, :], st2[:].rearrange("p (b o) c s -> p o b c s", o=2)[:, oh])
            nc.vector.tensor_scalar(ar3b[:, oh:oh + 1], mv2[:, oh, 0:1], 1536.0, None, ALU.mult)
            nc.vector.scalar_tensor_tensor(ar3b[:, 2 + oh:3 + oh], mv2[:, oh, 0:1],
                                           mv2[:, oh, 0:1], mv2[:, oh, 1:2], ALU.mult, ALU.add)
            nc.vector.tensor_scalar(ar3b[:, 2 + oh:3 + oh], ar3b[:, 2 + oh:3 + oh],
                                    1536.0, None, ALU.mult)
        ar3_i = dram.tile([128, 4], f32)
        ar3_o = dram.tile([NCORES, 128, 4], f32)
        nc.sync.dma_start(ar3_i[:], ar3b[:])
        nc.gpsimd.collective_compute("AllGather", mybir.AluOpType.bypass,
                                     replica_groups=[list(range(NCORES))],
                                     ins=[ar3_i.opt()], outs=[ar3_o.opt()])
        ar3g = small.tile([128, NCORES, 4], f32)
        nc.scalar.dma_start(ar3g[:], ar3_o[:].rearrange("d p f -> p d f"))
        ar3r = small.tile([128, 4], f32)
        nc.vector.tensor_reduce(ar3r[:], ar3g[:].rearrange("p d f -> p f d"),
                                mybir.AxisListType.X, ALU.add)

        # -------- b=1 groups fill the AR3 latency --------
        for g in range(NT):
            emit_exp_group(1, g, e2ps, csb)
        for g in range(NT):
            emit_transpose_group(1, g)

        # -------- BN2 apply + max + y2 --------
        a2 = small.tile([128, 2], f32); b2 = small.tile([128, 2], f32)
        v = nc.vector
        v.tensor_scalar(wk2[:, 0:2], ar3r[:, 0:2], 1.0 / CNT1, None, ALU.mult)  # means
        v.tensor_scalar(wk2[:, 2:4], ar3r[:, 2:4], 1.0 / CNT1, None, ALU.mult)  # E[x^2]
        v.tensor_tensor(wk2[:, 4:6], wk2[:, 0:2], wk2[:, 0:2], ALU.mult)        # m^2
        v.tensor_tensor(wk2[:, 2:4], wk2[:, 2:4], wk2[:, 4:6], ALU.subtract)    # var
        v.tensor_scalar(wk2[:, 2:4], wk2[:, 2:4], BNE, None, ALU.add)           # var+eps
        emit_rsqrt(wk2[:, 4:6], wk2[:, 2:4], 2)
        v.tensor_tensor(a2[:, 0:2], wk2[:, 4:6], pp[:, G2_:G2_ + 2], ALU.mult)
        v.tensor_tensor(wk2[:, 0:2], wk2[:, 0:2], a2[:, 0:2], ALU.mult)         # m*a2
        v.tensor_tensor(b2[:, 0:2], pp[:, BB2:BB2 + 2], wk2[:, 0:2], ALU.subtract)
        y2 = [small.tile([128, 128], bf16, tag=f"y2_{b}_{kh}", name=f"y2_{b}_{kh}")
              for b in range(BLOC) for kh in range(2)]
        if True:
            dps = cps
            for b in range(BLOC):
                gT = csb.tile([128, K], bf16, tag="gT", name=f"gT_{b}")
                for oh in range(2):
                    if fold2:
                        grl = csb.tile([128, 128], bf16, tag="grl", name=f"grl_{b}_{oh}")
                        nc.scalar.activation(grl[:], gmxr[(b, oh)][:], AF.Relu,
                                             bias=b2[:, oh:oh + 1], scale=a2[:, oh:oh + 1])
                    else:
                        t2 = csb.tile([128, 6, 128], f32, tag="t2", name=f"t2_{b}_{oh}")
                        nc.scalar.activation(t2[:].rearrange("p a b -> p (a b)"),
                                             c2[b][oh][:].rearrange("p a b -> p (a b)"),
                                             AF.Identity, bias=b2[:, oh:oh + 1], scale=a2[:, oh:oh + 1])
                        gmx = csb.tile([128, 128], f32, tag="gmx", name=f"gmx_{b}_{oh}")
                        nc.vector.reduce_max(gmx[:], t2[:].rearrange("p a b -> p b a"),
                                             mybir.AxisListType.X)
                        grl = csb.tile([128, 128], bf16, tag="grl", name=f"grl_{b}_{oh}")
                        nc.scalar.activation(grl[:], gmx[:], AF.Relu)
                    gtp2 = dps.tile([128, 128], bf16, tag="kmtp", bufs=1, name=f"gtp2_{b}_{oh}")
                    nc.tensor.transpose(gtp2[:], grl[:], idb[:])
                    nc.vector.tensor_copy(gT[:, oh * 128:(oh + 1) * 128], gtp2[:])
                for kh in range(2):
                    yp = dps.tile([128, 128], f32, tag="kntp", bufs=1, name=f"yp_{b}_{kh}")
                    nc.tensor.matmul(yp[:], gT[:, kh * 128:(kh + 1) * 128], scB[:])
                    nc.vector.tensor_copy(y2[b * 2 + kh][:], yp[:])

    # ---------------- final: out = scA@x + y2@S2 + sc_b ----------------
    with (
        tc.tile_pool(name="ph_e_ps", bufs=1, space="PSUM") as fps_p,
        tc.tile_pool(name="ph_e_sb", bufs=2) as esb,
    ):
        for t0 in range(0, NT, 4):
            emit_final_block(0, list(range(t0, t0 + 4)), fps_p)
            emit_final_block(1, list(range(t0, t0 + 4)), fps_p)
    es.close()


def _build(cfg, iters: int = 1):
    key = (cfg, iters)
    if key in _cache:
        return _cache[key]
    nc = bacc.Bacc("TRN2", target_bir_lowering=False, debug=False,
                   num_devices=NCORES)
    with tile.TileContext(nc) as tc:
        _emit(nc, tc, cfg, iters=iters)
    nc.compile()
    _cache[key] = nc
    return nc


def prepare(inputs):
    """Host-side prep: returns (cfg, in_maps)."""
    x = np.asarray(inputs['data'])[..., 0]          # [B, C, N] f32
    dp_gamma = np.asarray(inputs['dp_gamma']); dp_beta = np.asarray(inputs['dp_beta'])
    du_gamma = np.asarray(inputs['du_gamma']); du_beta = np.asarray(inputs['du_beta'])
    dp_w = np.asarray(inputs['dp_w']); du_w = np.asarray(inputs['du_w'])
    du_b = np.asarray(inputs['du_b'])
    dg_w1 = np.asarray(inputs['dg_w1']); dg_w2 = np.asarray(inputs['dg_w2'])
    dg_g1 = np.asarray(inputs['dg_g1']); dg_bb1 = np.asarray(inputs['dg_bb1'])
    dg_g2 = np.asarray(inputs['dg_g2']); dg_bb2 = np.asarray(inputs['dg_bb2'])
    sc_w = np.asarray(inputs['sc_w']); sc_b = np.asarray(inputs['sc_b'])

    fold = bool((dp_gamma > 0).all() and (du_gamma > 0).all()
                and not dp_beta.any() and not du_beta.any())
    reuse_h = bool(np.array_equal(dp_gamma, du_gamma) and np.array_equal(dp_beta, du_beta))
    dub_zero = bool(np.all(du_b == 0))
    fold2 = bool((dg_g2 > 0).all())
    cfg = (fold, reuse_h, dub_zero, fold2)

    bf = ml_dtypes.bfloat16
    x_bf = x.astype(bf)
    pp = np.zeros((128, 13), np.float32)
    pp[:, 0] = dp_gamma; pp[:, 1] = dp_beta; pp[:, 2] = du_gamma; pp[:, 3] = du_beta
    pp[:, 4] = sc_b
    for hh in range(2):
        sl = slice(hh * 128, (hh + 1) * 128)
        pp[:, 5 + hh] = dg_g1[sl]; pp[:, 7 + hh] = dg_bb1[sl]
        pp[:, 9 + hh] = dg_g2[sl]; pp[:, 11 + hh] = dg_bb2[sl]

    const_map = {
        "dpw_t": np.ascontiguousarray(dp_w.T).astype(bf),
        "duw_t": np.ascontiguousarray(du_w.T).astype(bf),
        "scA_t": np.ascontiguousarray(sc_w[:, :128].T).astype(bf),
        "scB_t": np.ascontiguousarray(sc_w[:, 128:].T).astype(bf),
        "w1s_t": np.ascontiguousarray((dg_w1[:, :256] + dg_w1[:, 256:]).T).astype(bf),
        "w1b_t": np.ascontiguousarray(dg_w1[:, 256:].T).astype(bf),
        "w2_t": np.ascontiguousarray(dg_w2.T).astype(bf),
        "identf": np.eye(128, dtype=np.float32),
        "identb": np.eye(128).astype(bf),
        "pp": pp,
        "dub_bc": np.broadcast_to(du_b[None, :], (128, 256)).astype(np.float32).copy(),
    }
    in_maps = []
    for c in range(NCORES):
        m = dict(const_map)
        xv = x_bf[c * BLOC:(c + 1) * BLOC]
        m["xb"] = np.ascontiguousarray(xv)
        m["xbT"] = np.ascontiguousarray(
            xv.reshape(BLOC, C, N // 128, 128).transpose(3, 0, 2, 1))
        in_maps.append(m)
    return cfg, in_maps


def kernel(**inputs) -> np.ndarray:
    cfg, in_maps = prepare(inputs)
    nc = _build(cfg, ITERS)
    global LAST_RESULT
    res = run_bass_kernel_spmd(nc, in_maps, core_ids=list(range(NCORES)),
                               trace=TRACE, **(TRACE_KW or {}))
    LAST_RESULT = res
    out = np.concatenate([res.results[c]["out"] for c in range(NCORES)], axis=0)
    return out[..., None].astype(np.float32)


if __name__ == "__main__":
    import reference
    ins = {k: np.asarray(v) for k, v in reference.setup_inputs().items()}
    got = kernel(**ins)
    print("out shape", got.shape, got.dtype)

